# revision 17
# baseline (speedup 1.0000x reference)
"""Trainium2 8-core kernel for nn_Attention_68341519614426.

Reference computation (B=4, N=2048, D=1024, H=16, pd=64):
    qkv = x @ Wqkv.T + bqkv                       # (B, N, 3D)
    q, k, v = split/reshape -> (B, H, N, pd)
    att = softmax(q @ k.T)  (NO 1/sqrt(pd) scale)
    out = (att @ v)  reshaped (B,H,N,pd) -> (B,N,D) with NO transpose,
    i.e. each (b, h) head's flattened (N, pd) block is a contiguous chunk
    of the output.  => 64 fully independent (b, h) problems.

Sharding: 8 cores = 4 batches x 2 head-groups (8 heads each).  Pure data
parallel, no collectives.  Host pre-transposes/casts inputs; device does
QKV projection, scores, softmax (exp + fused denominator via an appended
ones-column of V), att@v, transpose back to token-major, normalization.

Device dataflow (per core, per HEAD-PAIR g = heads (2g, 2g+1)):
  qT,kT  : feature-major  [pd-feat (partitions), tokens]   (bf16)
           head 2g on partitions 0-63, head 2g+1 on partitions 64-127.
  v_aug  : token-major    [tokens (partitions), 64 v feats + ones col]
  S^T    : [128 key-chunk, 2 heads, 512 queries] psum -- the two heads'
           scores matmuls are K=64 and auto-derive tile_position (0,0) /
           (64,0), so the PE runs them CONCURRENTLY on disjoint row
           strips (full-array utilization instead of half).
  E^T=exp(S^T): ONE ScalarE activation per iteration covers both heads
           (N=1024 free elements) -- ScalarE is the envelope engine at
           (N+352)/1.2 ns per instruction.
  O_aug^T[65, 512] (psum, per head) += v_aug[m].T @ E^T[m]  (row 64 =
           softmax denominator via the ones column; M=65 blocks column
           pairing, which is why only the scores leg is paired).
  PE-transpose 128-token blocks -> [128 tok, 65], DVE divide by denom,
  DMA out.

Scheduling notes (HW-measured on trn2):
  * ScalarE exp is the steady-state envelope (~1147 ns per [128,2,512]
    tile pair vs ~640 ns of PE work), so QKV projection matmuls for
    later head-pairs (and junk full-array matmuls once those run out)
    fill the PE idle inside the attention loop, which also keeps the
    PE HAM clock gate at K=8/8 (it re-throttles 2.4 -> 1.2 GHz after a
    ~3.4 us idle window and recovers slowly under half-array streams).
  * The att@v matmuls for iteration i are emitted during iteration i+1
    (software pipeline): the in-order PE queue must not park an
    exp-dependent matmul in front of the next scores matmul.
  * Each 512-query block's flush/transpose/normalize epilogue is
    deferred into the next block's iterations via a step deque.
  * PSUM budget (8 banks): 'st' tag ring 3 x [128,2,512]f32 (6 banks,
    shared by scores tiles, QKV-filler tiles and transpose targets) +
    o_tA/o_tB [65,512]f32 (1 bank each).
"""

import os
import sys
from collections import deque

import numpy as np

if "/opt/trn_rl_repo" not in sys.path:
    sys.path.insert(0, "/opt/trn_rl_repo")

import ml_dtypes

import concourse.bass as bass
import concourse.tile as tile
from concourse import bacc, mybir
from concourse.bass_utils import run_bass_kernel_spmd
from concourse.masks import make_identity

BF16 = ml_dtypes.bfloat16

B, N, D = 4, 2048, 1024
H = 16
PD = 64
HEADS_PER_CORE = 8  # 2-way head parallel x 4-way batch parallel
SHARD_F = HEADS_PER_CORE * PD  # 512 q (or k, or v) features per core

_CACHE = {}


def _build_nc() -> bass.Bass:
    f32 = mybir.dt.float32
    bf16 = mybir.dt.bfloat16

    nc = bacc.Bacc()
    xt_h = nc.declare_dram_parameter("xt", [D, N], bf16, isOutput=False)
    wt_h = nc.declare_dram_parameter("wt", [D, 3 * SHARD_F], bf16, isOutput=False)
    bqk_h = nc.declare_dram_parameter("bias_qk", [128, 8], f32, isOutput=False)
    bv_h = nc.declare_dram_parameter(
        "bias_v", [128, HEADS_PER_CORE, PD], f32, isOutput=False
    )
    # Feature-major output [head, pd, N]: the host transposes each head's
    # [pd, N] block to token-major during the unshard gather.  This removes
    # all 128 PE-transposes and the per-chunk recip/mul epilogue from the
    # device critical path; normalization (the softmax divide) stays on
    # device, done feature-major against a partition-broadcast reciprocal
    # denominator row.
    out_h = nc.declare_dram_parameter(
        "out", [HEADS_PER_CORE, PD, N], f32, isOutput=True
    )

    KC = D // 128  # 8 contraction chunks for the QKV projection
    NT512 = N // 512  # 4
    MCH = N // 128  # 16 key-token chunks
    QC = SHARD_F // 128  # 4 feature chunks for q (and for k)
    NPAIR = HEADS_PER_CORE // 2  # 4 head pairs

    with tile.TileContext(nc) as tc:
        with (
            tc.tile_pool(name="consts", bufs=1) as consts,
            tc.tile_pool(name="big", bufs=1) as big,
            tc.tile_pool(name="ps", bufs=2, space="PSUM") as ps,
            tc.tile_pool(name="scr", bufs=2, space="PSUM") as scr,
            tc.tile_pool(name="ops", bufs=1, space="PSUM") as ops,
            tc.tile_pool(name="epool", bufs=5) as epool,
            tc.tile_pool(name="onorm", bufs=2) as onorm,
            tc.tile_pool(name="rbp", bufs=4) as rbp,
        ):
            # ---- constants / inputs resident in SBUF ----
            bqk_sb = consts.tile([128, 8], f32, tag="bqk")
            nc.sync.dma_start(out=bqk_sb, in_=bqk_h[:])
            bv_sb = consts.tile([128, HEADS_PER_CORE, PD], f32, tag="bv")
            nc.sync.dma_start(out=bv_sb, in_=bv_h[:])

            # per-chunk input DMAs: spread across DMA engines so the
            # first projection matmuls start ~2us in instead of waiting on
            # one serialized multi-MB transfer
            xt_sb = big.tile([128, KC, N], bf16, tag="xt")
            wt_sb = big.tile([128, KC, 3 * SHARD_F], bf16, tag="wt")
            for kc in range(KC):
                nc.sync.dma_start(
                    out=wt_sb[:, kc, 2 * SHARD_F : 3 * SHARD_F],
                    in_=wt_h[kc * 128 : (kc + 1) * 128, 2 * SHARD_F : 3 * SHARD_F],
                )
                nc.sync.dma_start(
                    out=xt_sb[:, kc, :], in_=xt_h[kc * 128 : (kc + 1) * 128, :]
                )
            for kc in range(KC):
                nc.sync.dma_start(
                    out=wt_sb[:, kc, 0 : 2 * SHARD_F],
                    in_=wt_h[kc * 128 : (kc + 1) * 128, 0 : 2 * SHARD_F],
                )

            qt_sb = big.tile([128, QC, N], bf16, tag="qt")
            kt_sb = big.tile([128, QC, N], bf16, tag="kt")
            vaug_sb = big.tile([128, MCH, HEADS_PER_CORE, PD + 1], bf16, tag="vaug")
            nc.vector.memset(vaug_sb[:, :, :, PD : PD + 1], 1.0)

            def qk_psum():
                # Dedicated scratch bank ring: a projection tile accumulates
                # across 8 pe_filler() calls spread over many iterations, so
                # it must NOT share the scores 'st' ring (slot reuse would
                # clobber the in-progress accumulation).
                return scr.tile([128, 512], f32, tag="scr", name="qkscr")

            def emit_qk_tile(fc, t5):
                """One q/k projection psum tile: 8 matmuls + bias drain."""
                dst = qt_sb if fc < QC else kt_sb
                cc = fc % QC
                pt = qk_psum()
                for kc in range(KC):
                    nc.tensor.matmul(
                        pt,
                        lhsT=wt_sb[:, kc, fc * 128 : (fc + 1) * 128],
                        rhs=xt_sb[:, kc, t5 * 512 : (t5 + 1) * 512],
                        start=(kc == 0),
                        stop=(kc == KC - 1),
                    )
                nc.vector.tensor_scalar_add(
                    dst[:, cc, t5 * 512 : (t5 + 1) * 512],
                    pt,
                    bqk_sb[:, fc : fc + 1],
                )

            def qk_mm_gen(chunks):
                """Generator: one q/k projection matmul per next() call."""
                for c in chunks:
                    for fc in (c, QC + c):  # q chunk c, then k chunk c
                        dst = qt_sb if fc < QC else kt_sb
                        cc = fc % QC
                        for t5 in range(NT512):
                            pt = qk_psum()
                            for kc in range(KC):
                                nc.tensor.matmul(
                                    pt,
                                    lhsT=wt_sb[:, kc, fc * 128 : (fc + 1) * 128],
                                    rhs=xt_sb[:, kc, t5 * 512 : (t5 + 1) * 512],
                                    start=(kc == 0),
                                    stop=(kc == KC - 1),
                                )
                                if kc == KC - 1:
                                    nc.vector.tensor_scalar_add(
                                        dst[:, cc, t5 * 512 : (t5 + 1) * 512],
                                        pt,
                                        bqk_sb[:, fc : fc + 1],
                                    )
                                yield True

            # ---- stage 1 preamble: v projection (token-major) + qk chunk 0 ----
            with nc.named_scope("qkv_preamble"):
                for tk in range(MCH):
                    pt = qk_psum()
                    for kc in range(KC):
                        nc.tensor.matmul(
                            pt,
                            lhsT=xt_sb[:, kc, tk * 128 : (tk + 1) * 128],
                            rhs=wt_sb[:, kc, 2 * SHARD_F : 3 * SHARD_F],
                            start=(kc == 0),
                            stop=(kc == KC - 1),
                        )
                    nc.vector.tensor_add(
                        vaug_sb[:, tk, :, 0:PD],
                        pt.rearrange("p (h j) -> p h j", j=PD),
                        bv_sb,
                    )
                for fc in (0, QC):  # q chunk 0, k chunk 0
                    for t5 in range(NT512):
                        emit_qk_tile(fc, t5)

            # remaining q/k work, interleaved into the attention loops
            qk_fill = qk_mm_gen([1, 2, 3])

            fill_state = {"mms": 0, "pause": False}

            def pe_filler():
                """Interleave one q/k projection matmul into the PE stream.

                Real work only: once the projection is done this is a no-op.
                (The HAM clock gate stays warm without junk matmuls now --
                per-iteration PE idle is far below the ~3.4us MID window.)
                After each completed projection tile (8 matmuls) one call is
                skipped so the DVE bias-drain can free the psum slot without
                stalling the PE.
                """
                if fill_state["pause"]:
                    fill_state["pause"] = False
                    return
                if next(qk_fill, None) is not None:
                    fill_state["mms"] += 1
                    if fill_state["mms"] % 8 == 0:
                        fill_state["pause"] = True

            # Deferred epilogues: each 512-query block's normalize chain
            # (reciprocal of the denominator row, partition-broadcast,
            # feature-major multiply, final DMA) is queued and consumed two
            # steps per subsequent inner-loop iteration.  The epilogue is
            # appended only at iteration 3 of the NEXT block, after the
            # lag-3 att@v pipeline has emitted all of the previous block's
            # accumulation matmuls (emission order defines Tile deps).
            epilogue = deque()

            def epi_step():
                if epilogue:
                    epilogue.popleft()()

            def emit_attv(p):
                p_et, p_m, p_oA, p_oB, p_hA = p
                nc.tensor.matmul(
                    p_oA,
                    lhsT=vaug_sb[:, p_m, p_hA, :],
                    rhs=p_et[:, 0, :],
                    start=(p_m == 0),
                    stop=(p_m == MCH - 1),
                )
                nc.tensor.matmul(
                    p_oB,
                    lhsT=vaug_sb[:, p_m, p_hA + 1, :],
                    rhs=p_et[:, 1, :],
                    start=(p_m == 0),
                    stop=(p_m == MCH - 1),
                )

            # ---- stage 2: per-head-pair attention ----
            pend = deque()  # (et, m, o_tA, o_tB, hA): att@v lags 3 iters
            pending_epi = []
            for g in range(NPAIR):
                hA = 2 * g
                o_nA = onorm.tile([PD, N], f32, tag="onA")
                o_nB = onorm.tile([PD, N], f32, tag="onB")
                for nh in range(NT512):
                    nsl = nh * 512
                    o_tA = ops.tile([65, 512], f32, tag="OA")
                    o_tB = ops.tile([65, 512], f32, tag="OB")
                    for m in range(MCH):
                        st = ps.tile([128, 2, 512], f32, tag="st")
                        # two heads' scores: K=64 row strips (0,0) / (64,0)
                        # -> concurrent on the PE
                        nc.tensor.matmul(
                            st[:, 0, :],
                            lhsT=kt_sb[0:PD, g, m * 128 : (m + 1) * 128],
                            rhs=qt_sb[0:PD, g, nsl : nsl + 512],
                            start=True,
                            stop=True,
                        )
                        nc.tensor.matmul(
                            st[:, 1, :],
                            lhsT=kt_sb[PD:128, g, m * 128 : (m + 1) * 128],
                            rhs=qt_sb[PD:128, g, nsl : nsl + 512],
                            start=True,
                            stop=True,
                        )
                        et = epool.tile([128, 2, 512], bf16, tag="E")
                        nc.scalar.activation(
                            out=et, in_=st, func=mybir.ActivationFunctionType.Exp
                        )
                        pend.append((et, m, o_tA, o_tB, hA))
                        if len(pend) > 3:
                            emit_attv(pend.popleft())
                        if m == 3 and pending_epi:
                            epilogue.extend(pending_epi)
                            pending_epi = []
                        epi_step()
                        epi_step()
                        pe_filler()
                        if g < 2:
                            pe_filler()

                    def make_epilogue(
                        o_tA=o_tA, o_tB=o_tB, o_nA=o_nA, o_nB=o_nB, nh=nh, hA=hA
                    ):
                        steps = []
                        for o_t, o_n in ((o_tA, o_nA), (o_tB, o_nB)):
                            def mk(o_t=o_t, o_n=o_n):
                                box = {}

                                def s_flush():
                                    # single DVE op frees the o_t psum bank
                                    # immediately (the next block's att@v
                                    # start=True waits only on this); the
                                    # rest of the normalize chain works from
                                    # the SBUF copy off the critical path
                                    box["of"] = rbp.tile(
                                        [65, 512], f32, tag="of", name="of", bufs=2
                                    )
                                    nc.vector.tensor_copy(box["of"], o_t)

                                def s_recip():
                                    box["rrow"] = rbp.tile(
                                        [1, 512], f32, tag="rrow", name="rrow"
                                    )
                                    nc.vector.reciprocal(
                                        box["rrow"], box["of"][PD : PD + 1, :]
                                    )

                                def s_bcast():
                                    box["rb"] = rbp.tile(
                                        [PD, 512], f32, tag="rb", name="rb"
                                    )
                                    nc.gpsimd.partition_broadcast(
                                        box["rb"], box["rrow"]
                                    )

                                def s_mul():
                                    nc.vector.tensor_mul(
                                        o_n[:, nh * 512 : (nh + 1) * 512],
                                        box["of"][0:PD, :],
                                        box["rb"],
                                    )

                                return [s_flush, s_recip, s_bcast, s_mul]

                            steps.extend(mk())
                        def dmaA():
                            nc.sync.dma_start(
                                out=out_h[hA, :, nh * 512 : (nh + 1) * 512],
                                in_=o_nA[:, nh * 512 : (nh + 1) * 512],
                            )

                        def dmaB():
                            nc.sync.dma_start(
                                out=out_h[hA + 1, :, nh * 512 : (nh + 1) * 512],
                                in_=o_nB[:, nh * 512 : (nh + 1) * 512],
                            )

                        steps.append(dmaA)
                        steps.append(dmaB)
                        return steps

                    pending_epi = make_epilogue()

            # drain: remaining att@v pairs, then the last epilogue steps
            while pend:
                emit_attv(pend.popleft())
            epilogue.extend(pending_epi)
            while epilogue:
                epi_step()
                pe_filler()
    nc.finalize()
    return nc


def _prep_core_inputs(x, Wqkv, bqkv, core):
    b, g = core // 2, core % 2
    xt = np.ascontiguousarray(x[b].T).astype(BF16)  # (D, N)
    wq = Wqkv[g * SHARD_F : (g + 1) * SHARD_F]
    wk = Wqkv[D + g * SHARD_F : D + (g + 1) * SHARD_F]
    wv = Wqkv[2 * D + g * SHARD_F : 2 * D + (g + 1) * SHARD_F]
    wt = np.ascontiguousarray(np.concatenate([wq, wk, wv], axis=0).T).astype(BF16)
    bq = bqkv[g * SHARD_F : (g + 1) * SHARD_F]
    bk = bqkv[D + g * SHARD_F : D + (g + 1) * SHARD_F]
    bv = bqkv[2 * D + g * SHARD_F : 2 * D + (g + 1) * SHARD_F]
    bias_qk = np.concatenate(
        [bq.reshape(4, 128).T, bk.reshape(4, 128).T], axis=1
    ).astype(np.float32)  # (128, 8)
    bias_v = np.broadcast_to(
        bv.reshape(HEADS_PER_CORE, PD), (128, HEADS_PER_CORE, PD)
    ).astype(np.float32)
    return {
        "xt": xt,
        "wt": wt,
        "bias_qk": np.ascontiguousarray(bias_qk),
        "bias_v": np.ascontiguousarray(bias_v),
    }


def kernel(x, Wqkv, bqkv):
    x = np.asarray(x, dtype=np.float32)
    Wqkv = np.asarray(Wqkv, dtype=np.float32)
    bqkv = np.asarray(bqkv, dtype=np.float32)

    if "nc" not in _CACHE:
        _CACHE["nc"] = _build_nc()
    nc = _CACHE["nc"]

    in_maps = [_prep_core_inputs(x, Wqkv, bqkv, c) for c in range(8)]
    res = run_bass_kernel_spmd(nc, in_maps, core_ids=list(range(8)))
    _CACHE["last_result"] = res

    full = np.empty((B, H, N * PD), dtype=np.float32)
    for c in range(8):
        b, g = c // 2, c % 2
        # device emits feature-major [head, PD, N]; unshard transposes each
        # head's block to token-major [N, PD]
        arr = res.results[c]["out"]  # (HEADS_PER_CORE, PD, N)
        full[b, g * HEADS_PER_CORE : (g + 1) * HEADS_PER_CORE] = (
            arr.transpose(0, 2, 1).reshape(HEADS_PER_CORE, N * PD)
        )
    return full.reshape(B, N, D)


# revision 19
# speedup vs baseline: 1.2170x; 1.2170x over previous
"""Trainium2 8-core kernel for nn_Attention_68341519614426.

Reference computation (B=4, N=2048, D=1024, H=16, pd=64):
    qkv = x @ Wqkv.T + bqkv                       # (B, N, 3D)
    q, k, v = split/reshape -> (B, H, N, pd)
    att = softmax(q @ k.T)  (NO 1/sqrt(pd) scale)
    out = (att @ v)  reshaped (B,H,N,pd) -> (B,N,D) with NO transpose,
    i.e. each (b, h) head's flattened (N, pd) block is a contiguous chunk
    of the output.  => 64 fully independent (b, h) problems.

Sharding: 8 cores = 4 batches x 2 head-groups (8 heads each).  Pure data
parallel, no collectives.  Host pre-transposes/casts inputs; device does
QKV projection, scores, softmax (exp + fused denominator via an appended
ones-column of V), att@v, transpose back to token-major, normalization.

Device dataflow (per core, per HEAD-PAIR g = heads (2g, 2g+1)):
  qT,kT  : feature-major  [pd-feat (partitions), tokens]   (bf16)
           head 2g on partitions 0-63, head 2g+1 on partitions 64-127.
  v_aug  : token-major    [tokens (partitions), 64 v feats + ones col]
  S^T    : [128 key-chunk, 2 heads, 512 queries] psum -- the two heads'
           scores matmuls are K=64 and auto-derive tile_position (0,0) /
           (64,0), so the PE runs them CONCURRENTLY on disjoint row
           strips (full-array utilization instead of half).
  E^T=exp(S^T): ONE ScalarE activation per iteration covers both heads
           (N=1024 free elements) -- ScalarE is the envelope engine at
           (N+352)/1.2 ns per instruction.
  O_aug^T[65, 512] (psum, per head) += v_aug[m].T @ E^T[m]  (row 64 =
           softmax denominator via the ones column; M=65 blocks column
           pairing, which is why only the scores leg is paired).
  PE-transpose 128-token blocks -> [128 tok, 65], DVE divide by denom,
  DMA out.

Scheduling notes (HW-measured on trn2):
  * ScalarE exp is the steady-state envelope (~1147 ns per [128,2,512]
    tile pair vs ~640 ns of PE work), so QKV projection matmuls for
    later head-pairs (and junk full-array matmuls once those run out)
    fill the PE idle inside the attention loop, which also keeps the
    PE HAM clock gate at K=8/8 (it re-throttles 2.4 -> 1.2 GHz after a
    ~3.4 us idle window and recovers slowly under half-array streams).
  * The att@v matmuls for iteration i are emitted during iteration i+1
    (software pipeline): the in-order PE queue must not park an
    exp-dependent matmul in front of the next scores matmul.
  * Each 512-query block's flush/transpose/normalize epilogue is
    deferred into the next block's iterations via a step deque.
  * PSUM budget (8 banks): 'st' tag ring 3 x [128,2,512]f32 (6 banks,
    shared by scores tiles, QKV-filler tiles and transpose targets) +
    o_tA/o_tB [65,512]f32 (1 bank each).
"""

import os
import sys
from collections import deque

import numpy as np

if "/opt/trn_rl_repo" not in sys.path:
    sys.path.insert(0, "/opt/trn_rl_repo")

import ml_dtypes

import concourse.bass as bass
import concourse.tile as tile
from concourse import bacc, mybir
from concourse.bass_utils import run_bass_kernel_spmd
from concourse.masks import make_identity

BF16 = ml_dtypes.bfloat16

B, N, D = 4, 2048, 1024
H = 16
PD = 64
HEADS_PER_CORE = 8  # 2-way head parallel x 4-way batch parallel
SHARD_F = HEADS_PER_CORE * PD  # 512 q (or k, or v) features per core

_CACHE = {}


def _build_nc() -> bass.Bass:
    f32 = mybir.dt.float32
    bf16 = mybir.dt.bfloat16

    nc = bacc.Bacc()
    xt_h = nc.declare_dram_parameter("xt", [D, N], bf16, isOutput=False)
    wt_h = nc.declare_dram_parameter("wt", [D, 3 * SHARD_F], bf16, isOutput=False)
    bqk_h = nc.declare_dram_parameter("bias_qk", [128, 8], f32, isOutput=False)
    bv_h = nc.declare_dram_parameter(
        "bias_v", [128, HEADS_PER_CORE, PD], f32, isOutput=False
    )
    # Feature-major output [head, pd, N]: the host transposes each head's
    # [pd, N] block to token-major during the unshard gather.  This removes
    # all 128 PE-transposes and the per-chunk recip/mul epilogue from the
    # device critical path; normalization (the softmax divide) stays on
    # device, done feature-major against a partition-broadcast reciprocal
    # denominator row.
    out_h = nc.declare_dram_parameter(
        "out", [HEADS_PER_CORE, PD, N], f32, isOutput=True
    )

    KC = D // 128  # 8 contraction chunks for the QKV projection
    NT512 = N // 512  # 4
    MCH = N // 128  # 16 key-token chunks
    QC = SHARD_F // 128  # 4 feature chunks for q (and for k)
    NPAIR = HEADS_PER_CORE // 2  # 4 head pairs

    with tile.TileContext(nc) as tc:
        with (
            tc.tile_pool(name="consts", bufs=1) as consts,
            tc.tile_pool(name="big", bufs=1) as big,
            tc.tile_pool(name="ps", bufs=2, space="PSUM") as ps,
            tc.tile_pool(name="scr", bufs=2, space="PSUM") as scr,
            tc.tile_pool(name="ops", bufs=1, space="PSUM") as ops,
            tc.tile_pool(name="epool", bufs=5) as epool,
            tc.tile_pool(name="onorm", bufs=2) as onorm,
            tc.tile_pool(name="rbp", bufs=4) as rbp,
        ):
            # ---- constants / inputs resident in SBUF ----
            bqk_sb = consts.tile([128, 8], f32, tag="bqk")
            nc.sync.dma_start(out=bqk_sb, in_=bqk_h[:])
            bv_sb = consts.tile([128, HEADS_PER_CORE, PD], f32, tag="bv")
            nc.sync.dma_start(out=bv_sb, in_=bv_h[:])

            # per-chunk input DMAs: spread across DMA engines so the
            # first projection matmuls start ~2us in instead of waiting on
            # one serialized multi-MB transfer
            xt_sb = big.tile([128, KC, N], bf16, tag="xt")
            wt_sb = big.tile([128, KC, 3 * SHARD_F], bf16, tag="wt")
            for kc in range(KC):
                nc.sync.dma_start(
                    out=wt_sb[:, kc, 2 * SHARD_F : 3 * SHARD_F],
                    in_=wt_h[kc * 128 : (kc + 1) * 128, 2 * SHARD_F : 3 * SHARD_F],
                )
                nc.sync.dma_start(
                    out=xt_sb[:, kc, :], in_=xt_h[kc * 128 : (kc + 1) * 128, :]
                )
            for kc in range(KC):
                nc.sync.dma_start(
                    out=wt_sb[:, kc, 0 : 2 * SHARD_F],
                    in_=wt_h[kc * 128 : (kc + 1) * 128, 0 : 2 * SHARD_F],
                )

            qt_sb = big.tile([128, QC, N], bf16, tag="qt")
            kt_sb = big.tile([128, QC, N], bf16, tag="kt")
            vaug_sb = big.tile([128, MCH, HEADS_PER_CORE, PD + 1], bf16, tag="vaug")
            nc.vector.memset(vaug_sb[:, :, :, PD : PD + 1], 1.0)

            def qk_psum():
                # Dedicated scratch bank ring: a projection tile accumulates
                # across 8 pe_filler() calls spread over many iterations, so
                # it must NOT share the scores 'st' ring (slot reuse would
                # clobber the in-progress accumulation).
                return scr.tile([128, 512], f32, tag="scr", name="qkscr")

            def emit_qk_tile(fc, t5):
                """One q/k projection psum tile: 8 matmuls + bias drain."""
                dst = qt_sb if fc < QC else kt_sb
                cc = fc % QC
                pt = qk_psum()
                for kc in range(KC):
                    nc.tensor.matmul(
                        pt,
                        lhsT=wt_sb[:, kc, fc * 128 : (fc + 1) * 128],
                        rhs=xt_sb[:, kc, t5 * 512 : (t5 + 1) * 512],
                        start=(kc == 0),
                        stop=(kc == KC - 1),
                    )
                nc.vector.tensor_scalar_add(
                    dst[:, cc, t5 * 512 : (t5 + 1) * 512],
                    pt,
                    bqk_sb[:, fc : fc + 1],
                )

            def qk_mm_gen(chunks):
                """Generator: one q/k projection matmul per next() call."""
                for c in chunks:
                    for fc in (c, QC + c):  # q chunk c, then k chunk c
                        dst = qt_sb if fc < QC else kt_sb
                        cc = fc % QC
                        for t5 in range(NT512):
                            pt = qk_psum()
                            for kc in range(KC):
                                nc.tensor.matmul(
                                    pt,
                                    lhsT=wt_sb[:, kc, fc * 128 : (fc + 1) * 128],
                                    rhs=xt_sb[:, kc, t5 * 512 : (t5 + 1) * 512],
                                    start=(kc == 0),
                                    stop=(kc == KC - 1),
                                )
                                if kc == KC - 1:
                                    nc.vector.tensor_scalar_add(
                                        dst[:, cc, t5 * 512 : (t5 + 1) * 512],
                                        pt,
                                        bqk_sb[:, fc : fc + 1],
                                    )
                                yield True

            # ---- stage 1 preamble: v projection (token-major) + qk chunk 0 ----
            with nc.named_scope("qkv_preamble"):
                for tk in range(MCH):
                    pt = qk_psum()
                    for kc in range(KC):
                        nc.tensor.matmul(
                            pt,
                            lhsT=xt_sb[:, kc, tk * 128 : (tk + 1) * 128],
                            rhs=wt_sb[:, kc, 2 * SHARD_F : 3 * SHARD_F],
                            start=(kc == 0),
                            stop=(kc == KC - 1),
                        )
                    nc.vector.tensor_add(
                        vaug_sb[:, tk, :, 0:PD],
                        pt.rearrange("p (h j) -> p h j", j=PD),
                        bv_sb,
                    )
                for fc in (0, QC):  # q chunk 0, k chunk 0
                    for t5 in range(NT512):
                        emit_qk_tile(fc, t5)

            # remaining q/k work, interleaved into the attention loops
            qk_fill = qk_mm_gen([1, 2, 3])

            fill_state = {"mms": 0, "pause": False}

            def pe_filler():
                """Interleave one q/k projection matmul into the PE stream.

                Real work only: once the projection is done this is a no-op.
                (The HAM clock gate stays warm without junk matmuls now --
                per-iteration PE idle is far below the ~3.4us MID window.)
                After each completed projection tile (8 matmuls) one call is
                skipped so the DVE bias-drain can free the psum slot without
                stalling the PE.
                """
                if fill_state["pause"]:
                    fill_state["pause"] = False
                    return
                if next(qk_fill, None) is not None:
                    fill_state["mms"] += 1
                    if fill_state["mms"] % 8 == 0:
                        fill_state["pause"] = True

            # Deferred epilogues: each 512-query block's normalize chain
            # (reciprocal of the denominator row, partition-broadcast,
            # feature-major multiply, final DMA) is queued and consumed two
            # steps per subsequent inner-loop iteration.  The epilogue is
            # appended only at iteration 3 of the NEXT block, after the
            # lag-3 att@v pipeline has emitted all of the previous block's
            # accumulation matmuls (emission order defines Tile deps).
            epilogue = deque()

            def epi_step():
                if epilogue:
                    epilogue.popleft()()

            def emit_attv(p):
                p_et, p_m, p_oA, p_oB, p_hA = p
                nc.tensor.matmul(
                    p_oA,
                    lhsT=vaug_sb[:, p_m, p_hA, :],
                    rhs=p_et[:, 0, :],
                    start=(p_m == 0),
                    stop=(p_m == MCH - 1),
                )
                nc.tensor.matmul(
                    p_oB,
                    lhsT=vaug_sb[:, p_m, p_hA + 1, :],
                    rhs=p_et[:, 1, :],
                    start=(p_m == 0),
                    stop=(p_m == MCH - 1),
                )

            # ---- stage 2: per-head-pair attention ----
            pend = deque()  # (et, m, o_tA, o_tB, hA): att@v lags 3 iters
            pending_epi = []
            for g in range(NPAIR):
                hA = 2 * g
                o_nA = onorm.tile([PD, N], f32, tag="onA")
                o_nB = onorm.tile([PD, N], f32, tag="onB")
                for nh in range(NT512):
                    nsl = nh * 512
                    o_tA = ops.tile([65, 512], f32, tag="OA")
                    o_tB = ops.tile([65, 512], f32, tag="OB")
                    for m in range(MCH):
                        st = ps.tile([128, 2, 512], f32, tag="st")
                        # two heads' scores: K=64 row strips (0,0) / (64,0)
                        # -> concurrent on the PE
                        nc.tensor.matmul(
                            st[:, 0, :],
                            lhsT=kt_sb[0:PD, g, m * 128 : (m + 1) * 128],
                            rhs=qt_sb[0:PD, g, nsl : nsl + 512],
                            start=True,
                            stop=True,
                        )
                        nc.tensor.matmul(
                            st[:, 1, :],
                            lhsT=kt_sb[PD:128, g, m * 128 : (m + 1) * 128],
                            rhs=qt_sb[PD:128, g, nsl : nsl + 512],
                            start=True,
                            stop=True,
                        )
                        et = epool.tile([128, 2, 512], bf16, tag="E")
                        nc.scalar.activation(
                            out=et, in_=st, func=mybir.ActivationFunctionType.Exp
                        )
                        pend.append((et, m, o_tA, o_tB, hA))
                        if len(pend) > 3:
                            emit_attv(pend.popleft())
                        if m == 3 and pending_epi:
                            epilogue.extend(pending_epi)
                            pending_epi = []
                        epi_step()
                        epi_step()
                        pe_filler()
                        if g < 2:
                            pe_filler()

                    def make_epilogue(
                        o_tA=o_tA, o_tB=o_tB, o_nA=o_nA, o_nB=o_nB, nh=nh, hA=hA
                    ):
                        # flushes for BOTH heads go first: the next block's
                        # att@v (start=True) reuses these psum banks and is
                        # emitted at its iter 3, so both banks must free as
                        # early as possible
                        flushes = []
                        rest = []
                        for o_t, o_n in ((o_tA, o_nA), (o_tB, o_nB)):
                            def mk(o_t=o_t, o_n=o_n):
                                box = {}

                                def s_flush():
                                    # single DVE op frees the o_t psum bank
                                    # immediately (the next block's att@v
                                    # start=True waits only on this); the
                                    # rest of the normalize chain works from
                                    # the SBUF copy off the critical path
                                    box["of"] = rbp.tile(
                                        [65, 512], f32, tag="of", name="of", bufs=2
                                    )
                                    nc.vector.tensor_copy(box["of"], o_t)

                                def s_recip():
                                    box["rrow"] = rbp.tile(
                                        [1, 512], f32, tag="rrow", name="rrow"
                                    )
                                    nc.vector.reciprocal(
                                        box["rrow"], box["of"][PD : PD + 1, :]
                                    )

                                def s_bcast():
                                    box["rb"] = rbp.tile(
                                        [PD, 512], f32, tag="rb", name="rb"
                                    )
                                    nc.gpsimd.partition_broadcast(
                                        box["rb"], box["rrow"]
                                    )

                                def s_mul():
                                    nc.vector.tensor_mul(
                                        o_n[:, nh * 512 : (nh + 1) * 512],
                                        box["of"][0:PD, :],
                                        box["rb"],
                                    )

                                return [s_flush, s_recip, s_bcast, s_mul]

                            fl, *rst = mk()
                            flushes.append(fl)
                            rest.extend(rst)
                        steps = flushes + rest
                        def dmaA():
                            nc.sync.dma_start(
                                out=out_h[hA, :, nh * 512 : (nh + 1) * 512],
                                in_=o_nA[:, nh * 512 : (nh + 1) * 512],
                            )

                        def dmaB():
                            nc.sync.dma_start(
                                out=out_h[hA + 1, :, nh * 512 : (nh + 1) * 512],
                                in_=o_nB[:, nh * 512 : (nh + 1) * 512],
                            )

                        steps.append(dmaA)
                        steps.append(dmaB)
                        return steps

                    pending_epi = make_epilogue()

            # drain: remaining att@v pairs, then the last epilogue steps
            while pend:
                emit_attv(pend.popleft())
            epilogue.extend(pending_epi)
            while epilogue:
                epi_step()
                pe_filler()
    nc.finalize()
    return nc


def _prep_core_inputs(x, Wqkv, bqkv, core):
    b, g = core // 2, core % 2
    xt = np.ascontiguousarray(x[b].T).astype(BF16)  # (D, N)
    wq = Wqkv[g * SHARD_F : (g + 1) * SHARD_F]
    wk = Wqkv[D + g * SHARD_F : D + (g + 1) * SHARD_F]
    wv = Wqkv[2 * D + g * SHARD_F : 2 * D + (g + 1) * SHARD_F]
    wt = np.ascontiguousarray(np.concatenate([wq, wk, wv], axis=0).T).astype(BF16)
    bq = bqkv[g * SHARD_F : (g + 1) * SHARD_F]
    bk = bqkv[D + g * SHARD_F : D + (g + 1) * SHARD_F]
    bv = bqkv[2 * D + g * SHARD_F : 2 * D + (g + 1) * SHARD_F]
    bias_qk = np.concatenate(
        [bq.reshape(4, 128).T, bk.reshape(4, 128).T], axis=1
    ).astype(np.float32)  # (128, 8)
    bias_v = np.broadcast_to(
        bv.reshape(HEADS_PER_CORE, PD), (128, HEADS_PER_CORE, PD)
    ).astype(np.float32)
    return {
        "xt": xt,
        "wt": wt,
        "bias_qk": np.ascontiguousarray(bias_qk),
        "bias_v": np.ascontiguousarray(bias_v),
    }


def kernel(x, Wqkv, bqkv):
    x = np.asarray(x, dtype=np.float32)
    Wqkv = np.asarray(Wqkv, dtype=np.float32)
    bqkv = np.asarray(bqkv, dtype=np.float32)

    if "nc" not in _CACHE:
        _CACHE["nc"] = _build_nc()
    nc = _CACHE["nc"]

    in_maps = [_prep_core_inputs(x, Wqkv, bqkv, c) for c in range(8)]
    res = run_bass_kernel_spmd(nc, in_maps, core_ids=list(range(8)))
    _CACHE["last_result"] = res

    full = np.empty((B, H, N * PD), dtype=np.float32)
    for c in range(8):
        b, g = c // 2, c % 2
        # device emits feature-major [head, PD, N]; unshard transposes each
        # head's block to token-major [N, PD]
        arr = res.results[c]["out"]  # (HEADS_PER_CORE, PD, N)
        full[b, g * HEADS_PER_CORE : (g + 1) * HEADS_PER_CORE] = (
            arr.transpose(0, 2, 1).reshape(HEADS_PER_CORE, N * PD)
        )
    return full.reshape(B, N, D)
